# revision 21
# baseline (speedup 1.0000x reference)
"""MoE top-2 routed FFN (B=4, S=2048, D=1024, H=2048, E=8) on 8 TRN2 NeuronCores.

Strategy (expert-parallel, matching the sharding hint):
  - Host computes the tiny gate (softmax top-2) and builds per-expert token
    lists ("all-to-all dispatch" done at the sharding step).
  - Core e receives the tokens routed to expert e (gathered, transposed,
    zero-padded to capacity C), plus expert e's weights pre-packed into the
    exact tile layouts the kernel consumes.
  - Each core runs a dense FFN  out = coef * ((relu(x@W1.T)^2 * (x@W3.T)) @ W2.T)
    over its C tokens.  All matmuls run in bf16 with fp32 PSUM accumulation
    (measured end-to-end rel err ~4e-3); coefficients and outputs stay fp32.
  - Host scatter-adds the per-expert outputs back ("combine").

Per-core kernel structure (single pass, weights read once):
  phase 1: for each of 16 H-tiles m: psA = W1m @ xT, psB = W3m @ xT (PSUM),
           gT[m] = relu(psA)^2 * psB  (DVE, bf16)   [H, C] layout
  phase 2: for each 128-token tile: out[tok, :] = (gT.T @ W2T) * coef  (PSUM->DVE->DRAM)
"""

import os
import sys

import numpy as np

if os.path.isdir("/opt/trn_rl_repo") and "/opt/trn_rl_repo" not in sys.path:
    sys.path.insert(0, "/opt/trn_rl_repo")

import ml_dtypes

import concourse.bacc as bacc
import concourse.mybir as mybir
from concourse.bass_utils import run_bass_kernel_spmd
from concourse.tile import TileContext

B, S, D, H, E = 4, 2048, 1024, 2048, 8
N = B * S
P = 128
KT = D // P   # 8 contraction tiles over D
MT = H // P   # 16 tiles over H

F32 = mybir.dt.float32
BF16 = mybir.dt.bfloat16
BF16_NP = ml_dtypes.bfloat16

# Set by test harness to capture profiling info.
TRACE = False
LAST_RESULTS = None


def _token_groups(c0, cw):
    """Split [c0, c0+cw) into moving-dim groups of at most 512."""
    groups = []
    rem = cw
    off = c0
    while rem > 0:
        if 512 < rem < 768:
            g = max(min(rem - 256, 512), 256)
        else:
            g = min(512, rem)
        groups.append((off, g))
        off += g
        rem -= g
    return groups


def build_kernel(C):
    TT = C // P
    nc = bacc.Bacc("TRN2", target_bir_lowering=False)

    xt = nc.dram_tensor("xt", [KT, P, C], BF16, kind="ExternalInput")
    w1p = nc.dram_tensor("w1p", [MT, P, KT * P], BF16, kind="ExternalInput")
    w3p = nc.dram_tensor("w3p", [MT, P, KT * P], BF16, kind="ExternalInput")
    w2p = nc.dram_tensor("w2p", [MT, P, D], BF16, kind="ExternalInput")
    cf = nc.dram_tensor("cf", [P, TT], F32, kind="ExternalInput")
    out = nc.dram_tensor("out", [TT, 2, P, 512], F32, kind="ExternalOutput")

    with TileContext(nc) as tc:
        with (
            tc.tile_pool(name="xt_pool", bufs=KT) as xt_pool,
            tc.tile_pool(name="g_pool", bufs=1) as g_pool,
            tc.tile_pool(name="w13_pool", bufs=2) as w13_pool,
            tc.tile_pool(name="w2_pool", bufs=MT) as w2_pool,
            tc.tile_pool(name="tmp_pool", bufs=2) as tmp_pool,
            tc.tile_pool(name="ob_pool", bufs=3) as ob_pool,
            tc.tile_pool(name="const_pool", bufs=1) as const_pool,
            tc.tile_pool(name="psAB", bufs=3, space="PSUM") as psAB_pool,
            tc.tile_pool(name="psO", bufs=2, space="PSUM") as psO_pool,
        ):
            # --- PE warmup: flip the HAM clock gate (1.2->2.4GHz) before the
            # first real matmul's data lands. Depends only on a local memset,
            # so it starts as soon as the PE sequencer is live. ---------------
            warm = const_pool.tile([P, 512], BF16, tag="warm")
            nc.any.memset(warm[:], 0.0)
            pswarm = psO_pool.tile([P, 512], F32, tag="psO", name="pswarm")
            for i in range(6):
                nc.tensor.matmul(pswarm[:], warm[:, :P], warm[:],
                                 start=(i == 0), stop=(i == 5))
            warmsink = const_pool.tile([P, 1], F32, tag="warmsink")
            nc.vector.tensor_scalar_mul(warmsink[:], pswarm[:, :1], 0.0)

            # --- constants / resident tensors -------------------------------
            cft = const_pool.tile([P, TT], F32, tag="cft")
            nc.sync.dma_start(cft[:], cf[:])

            # XT split into head (first token group, unblocks PE fast) + tail
            HEAD = min(512, C)
            xts_head, xts_tail = [], []

            def xt_slice(k, g0, gw):
                if g0 < HEAD:
                    assert g0 + gw <= HEAD
                    return xts_head[k][:, g0:g0 + gw]
                return xts_tail[k][:, g0 - HEAD:g0 - HEAD + gw]

            w2ts = []

            # --- phase 1: gT[h, tok] = relu(W1 @ xT)^2 * (W3 @ xT) ----------
            gts = []
            for m in range(MT):
                gt = g_pool.tile([P, C], BF16, tag=f"g{m}", name=f"g_{m}")
                gts.append(gt)

            for m in range(MT):
                w1t = w13_pool.tile([P, KT * P], BF16, tag="w1t",
                                    name=f"w1_{m}")
                nc.sync.dma_start(w1t[:], w1p[m])
                w3t = w13_pool.tile([P, KT * P], BF16, tag="w3t",
                                    name=f"w3_{m}")
                nc.sync.dma_start(w3t[:], w3p[m])
                if m == 0:
                    # DMA order: w1[m0], w3[m0] -> x heads -> x tails.
                    # The first psA matmuls start as soon as w1[m0]+xh[0]
                    # land (~7us); everything later streams under compute.
                    for k in range(KT):
                        xh = xt_pool.tile([P, HEAD], BF16, tag="xh",
                                          name=f"xth_{k}")
                        nc.sync.dma_start(xh[:], xt[k][:, :HEAD])
                        xts_head.append(xh)

                if m == 0 and C > HEAD:
                    # emit XT tail loads after m0 weights: PE starts on the
                    # head group while the tail streams in
                    for k in range(KT):
                        xtl = xt_pool.tile([P, C - HEAD], BF16, tag="xl",
                                           name=f"xtt_{k}")
                        nc.sync.dma_start(xtl[:], xt[k][:, HEAD:])
                        xts_tail.append(xtl)

                for g0, gw in _token_groups(0, C):
                    psA = psAB_pool.tile([P, 512], F32, tag="psA",
                                         name=f"psA_{m}_{g0}")
                    psB = psAB_pool.tile([P, 512], F32, tag="psB",
                                         name=f"psB_{m}_{g0}")
                    for k in range(KT):
                        nc.tensor.matmul(
                            psA[:, :gw],
                            w1t[:, k * P:(k + 1) * P],
                            xt_slice(k, g0, gw),
                            start=(k == 0),
                            stop=(k == KT - 1),
                        )
                    for k in range(KT):
                        nc.tensor.matmul(
                            psB[:, :gw],
                            w3t[:, k * P:(k + 1) * P],
                            xt_slice(k, g0, gw),
                            start=(k == 0),
                            stop=(k == KT - 1),
                        )
                    r = tmp_pool.tile([P, 512], F32, tag="r",
                                      name=f"r_{m}_{g0}")
                    nc.vector.tensor_relu(r[:, :gw], psA[:, :gw])
                    t2 = tmp_pool.tile([P, 512], F32, tag="t2",
                                       name=f"t2_{m}_{g0}")
                    nc.vector.tensor_mul(t2[:, :gw], r[:, :gw], r[:, :gw])
                    nc.vector.tensor_mul(
                        gts[m][:, g0:g0 + gw],
                        t2[:, :gw],
                        psB[:, :gw],
                    )

            # W2 loads emitted after phase-1 DMAs: they ride the idle DMA
            # tail of phase 1, well before phase 2 needs them, without
            # delaying PE start.
            for hk in range(MT):
                w2t = w2_pool.tile([P, D], BF16, tag="w2t", name=f"w2_{hk}")
                nc.sync.dma_start(w2t[:], w2p[hk])
                w2ts.append(w2t)

            # --- phase 2: out[tok, d] = coef * (g.T @ W2T) ------------------
            for t in range(TT):
                for dg in range(2):
                    pso = psO_pool.tile([P, 512], F32, tag="psO",
                                        name=f"psO_{t}_{dg}")
                    for hk in range(MT):
                        nc.tensor.matmul(
                            pso[:],
                            gts[hk][:, t * P:(t + 1) * P],
                            w2ts[hk][:, dg * 512:(dg + 1) * 512],
                            start=(hk == 0),
                            stop=(hk == MT - 1),
                        )
                    ob = ob_pool.tile([P, 512], F32, tag="ob",
                                      name=f"ob_{t}_{dg}")
                    nc.vector.tensor_scalar_mul(ob[:], pso[:],
                                                cft[:, t:t + 1])
                    nc.sync.dma_start(out[t, dg], ob[:])

    if not nc.is_finalized():
        nc.finalize()
    return nc


def kernel(x, W1, W2, W3, gate_w, gate_b):
    global LAST_RESULTS

    xf = np.ascontiguousarray(x.reshape(N, D).astype(np.float32, copy=False))

    # ---- gate: softmax + top-2 (tiny, done on host) ------------------------
    logits = xf @ gate_w.T.astype(np.float32) + gate_b.astype(np.float32)
    logits -= logits.max(axis=-1, keepdims=True)
    probs = np.exp(logits)
    probs /= probs.sum(axis=-1, keepdims=True)
    order = np.argsort(-probs, axis=-1, kind="stable")
    i1, i2 = order[:, 0], order[:, 1]
    ar = np.arange(N)
    p1, p2 = probs[ar, i1], probs[ar, i2]
    ps = p1 + p2
    c1, c2 = p1 / ps, p2 / ps

    idx_list, coef_list = [], []
    for e in range(E):
        m1 = i1 == e
        m2 = i2 == e
        ide = np.nonzero(m1 | m2)[0]
        ce = np.where(m1[ide], c1[ide], c2[ide]).astype(np.float32)
        idx_list.append(ide)
        coef_list.append(ce)

    nmax = max(len(i) for i in idx_list)
    C = max(((nmax + P - 1) // P) * P, 512)
    TT = C // P

    # ---- per-core input packing -------------------------------------------
    in_maps = []
    for e in range(E):
        ide, ce = idx_list[e], coef_list[e]
        ne = len(ide)

        xg = np.zeros((C, D), np.float32)
        xg[:ne] = xf[ide]
        xt_np = np.ascontiguousarray(xg.T).reshape(KT, P, C).astype(BF16_NP)

        w1e = np.asarray(W1[e], np.float32)  # [H, D]
        w3e = np.asarray(W3[e], np.float32)  # [H, D]
        w2e = np.asarray(W2[e], np.float32)  # [D, H]
        # [m, h, k, d] -> [m, d, k, h] : packed[m][d, k*128+h] = W1[m*128+h, k*128+d]
        w1p_np = np.ascontiguousarray(
            w1e.reshape(MT, P, KT, P).transpose(0, 3, 2, 1)
        ).reshape(MT, P, KT * P).astype(BF16_NP)
        w3p_np = np.ascontiguousarray(
            w3e.reshape(MT, P, KT, P).transpose(0, 3, 2, 1)
        ).reshape(MT, P, KT * P).astype(BF16_NP)
        # W2T[h, d] tiles: [hk, h, d]
        w2p_np = np.ascontiguousarray(w2e.T).reshape(MT, P, D).astype(BF16_NP)

        cfe = np.zeros(C, np.float32)
        cfe[:ne] = ce
        cf_np = np.ascontiguousarray(cfe.reshape(TT, P).T)

        in_maps.append(
            {"xt": xt_np, "w1p": w1p_np, "w3p": w3p_np, "w2p": w2p_np,
             "cf": cf_np}
        )

    # ---- build + run on 8 cores -------------------------------------------
    nc = build_kernel(C)
    res = None
    last_exc = None
    for attempt in range(3):
        try:
            res = run_bass_kernel_spmd(
                nc, in_maps, core_ids=list(range(E)),
                trace=TRACE and attempt == 0,
            )
            break
        except Exception as exc:  # transient device wedge / trace plumbing
            last_exc = exc
    if res is None:
        raise last_exc
    LAST_RESULTS = res

    # ---- combine ----------------------------------------------------------
    out = np.zeros((N, D), np.float32)
    for e in range(E):
        ide = idx_list[e]
        oe = res.results[e]["out"]  # [TT, 2, P, 512]
        oe = oe.transpose(0, 2, 1, 3).reshape(C, D)
        out[ide] += oe[: len(ide)]

    return out.reshape(B, S, D)
